# revision 27
# baseline (speedup 1.0000x reference)
"""Chamfer distance (CDLoss) Trainium2 kernel, v2.

Problem: prediction [4, 8192, 3], ground_truth [4, 8192, 3] (fp32).
For each batch: d2[n,m] = max(||p_n||^2 + ||g_m||^2 - 2 p.g, 0);
out[b] = sum_n min_m d2 / N + sum_m min_n d2 / M.

Strategy (8 NeuronCores): core c handles (batch = c//2, row-half = c%2),
i.e. a 4096 x 8192 slab of the distance matrix.

v2 design (all rates HW-measured via mb.py):
  - NEGATED distances: host builds ap=[px,py,pz,||p||^2,1],
    ag=[2gx,2gy,2gz,-1,-||g||^2] so one K=5 fp32 matmul emits
    -d2 tiles; every reduction is then a native MAX (enables pool_max).
  - PE: 16 matmuls [128,512] per row block into two 4-bank PSUM wide
    tiles [128,2048] (double buffered).
  - ScalarE: batched PSUM exit, one copy per wide tile f32->bf16
    (~2.6us per 2048 = 660ns/tile-equiv; FD=512 copies cost 940ns).
  - VectorE: running column-max fold, 16x tensor_tensor FD=512 bf16
    (~510ns each, 2x mode); row max via 8x pool_max w=1024 (variant
    "pool", ~700ns each at 4x) or 16x rowbuf TT (variant "ttrow")
    + one small tensor_reduce per row block.
Host: negate, clamp, fold 128 partitions / two halves, final sums.
"""

import numpy as np

_B = 4
_N = 8192  # points per cloud
_HALF = _N // 2  # rows per core
_RB = _HALF // 128  # 32 row blocks
_WIDE = 2048  # ScalarE exit width (4 PSUM banks)
_NW = _N // _WIDE  # 4 wide groups per row block
_NCORES = 8
_BIG = 1.0e30

_CACHED_NC = None
_RUNNERS = {}
_DEFAULT_VARIANT = "rt4"


def _build_nc(repeat=1, variant=None):
    if variant is None:
        variant = _DEFAULT_VARIANT
    import concourse.bacc as bacc
    import concourse.tile as tile
    from concourse import mybir

    f32 = mybir.dt.float32
    bf16 = mybir.dt.bfloat16
    MAX = mybir.AluOpType.max

    nc = bacc.Bacc("TRN2", target_bir_lowering=False, debug=False)

    if variant in ("rt4", "rt4b", "rt4c", "rt4h"):
        return _build_nc_rt4(nc, repeat, batched_reduce=(variant == "rt4b"),
                             early_pools=(variant == "rt4c"),
                             loop_hints=(variant == "rt4h"))

    ap_d = nc.dram_tensor("ap", [5, _HALF], f32, kind="ExternalInput")
    ag_d = nc.dram_tensor("ag", [5, _N], f32, kind="ExternalInput")
    rowparts_d = nc.dram_tensor("rowparts", [128, _RB], bf16, kind="ExternalOutput")
    colmax_d = nc.dram_tensor("colmax", [128, _N], bf16, kind="ExternalOutput")

    with tile.TileContext(nc) as tc:
        with (
            tc.tile_pool(name="singles", bufs=1) as singles,
            tc.tile_pool(name="slabs", bufs=6 if variant == "wslab" else 2) as slabs,
            tc.tile_pool(name="rpool", bufs=2) as rpool,
            tc.tile_pool(name="psum", bufs=2, space="PSUM") as pp,
        ):
            ap_s = singles.tile([5, _HALF], f32)
            nc.sync.dma_start(out=ap_s[:], in_=ap_d[:])
            ag_s = singles.tile([5, _N], f32)
            nc.sync.dma_start(out=ag_s[:], in_=ag_d[:])

            colmax_s = singles.tile([128, _N], bf16)
            nc.vector.memset(colmax_s[:], -_BIG)
            rowparts_s = singles.tile([128, _RB], bf16)

            def _body_wslab():
                # per-wide slab tiles: ScalarE writes tile w while DVE
                # drains tile w-1 (different SBUF tiles, finer pipeline)
                for rb in range(_RB):
                    lhsT = ap_s[:, rb * 128 : (rb + 1) * 128]
                    parts = rpool.tile([128, 8], bf16, tag="parts")
                    for w in range(_NW):
                        t = pp.tile([128, _WIDE], f32, tag="t")
                        for j in range(_WIDE // 512):
                            c0 = w * _WIDE + j * 512
                            nc.tensor.matmul(
                                t[:, j * 512 : (j + 1) * 512],
                                lhsT,
                                ag_s[:, c0 : c0 + 512],
                                start=True,
                                stop=True,
                            )
                        ws = slabs.tile([128, _WIDE], bf16, tag="ws")
                        nc.scalar.copy(ws[:], t[:])
                        for j in range(_WIDE // 512):
                            c0 = w * _WIDE + j * 512
                            cs = colmax_s[:, c0 : c0 + 512]
                            nc.vector.tensor_tensor(
                                cs, cs, ws[:, j * 512 : (j + 1) * 512], op=MAX
                            )
                        for q in range(2):
                            win = ws[:, q * 1024 : (q + 1) * 1024].rearrange(
                                "p (n w) -> p n w", w=1024
                            )
                            nc.vector.pool(
                                parts[:, 2 * w + q : 2 * w + q + 1], win,
                                func=mybir.PoolFunctionType.max,
                            )
                    nc.vector.tensor_reduce(
                        rowparts_s[:, rb : rb + 1], parts[:],
                        axis=mybir.AxisListType.X, op=MAX,
                    )

            def _body():
                if variant == "wslab":
                    return _body_wslab()
                for rb in range(_RB):
                    lhsT = ap_s[:, rb * 128 : (rb + 1) * 128]
                    slab = slabs.tile([128, _N], bf16, tag="slab")
                    for w in range(_NW):
                        t = pp.tile([128, _WIDE], f32, tag="t")
                        for j in range(_WIDE // 512):
                            c0 = w * _WIDE + j * 512
                            nc.tensor.matmul(
                                t[:, j * 512 : (j + 1) * 512],
                                lhsT,
                                ag_s[:, c0 : c0 + 512],
                                start=True,
                                stop=True,
                            )
                        # batched PSUM exit on ScalarE, f32 -> bf16
                        nc.scalar.copy(slab[:, w * _WIDE : (w + 1) * _WIDE], t[:])
                        # running column-max folds (DVE, 2x bf16)
                        fw = 1024 if variant == "flat" else 512
                        for j in range(_WIDE // fw):
                            c0 = w * _WIDE + j * fw
                            cs = colmax_s[:, c0 : c0 + fw]
                            if variant == "fused":
                                # fold straight from PSUM f32 (skips the
                                # SBUF slab read; PSUM port is separate)
                                nc.vector.tensor_tensor(
                                    cs, cs, t[:, j * fw : (j + 1) * fw], op=MAX
                                )
                            else:
                                nc.vector.tensor_tensor(
                                    cs, cs, slab[:, c0 : c0 + fw], op=MAX
                                )
                    # row max of this 128-row block
                    if variant in ("pool", "flat", "fused"):
                        parts = rpool.tile([128, 8], bf16, tag="parts")
                        for q in range(8):
                            win = slab[:, q * 1024 : (q + 1) * 1024].rearrange(
                                "p (n w) -> p n w", w=1024
                            )
                            nc.vector.pool(
                                parts[:, q : q + 1], win,
                                func=mybir.PoolFunctionType.max,
                            )
                        nc.vector.tensor_reduce(
                            rowparts_s[:, rb : rb + 1], parts[:],
                            axis=mybir.AxisListType.X, op=MAX,
                        )
                    else:  # "ttrow"
                        rowbuf = rpool.tile([128, 512], bf16, tag="rowbuf")
                        nc.vector.tensor_copy(rowbuf[:], slab[:, 0:512])
                        for j in range(1, 16):
                            nc.vector.tensor_tensor(
                                rowbuf[:], rowbuf[:],
                                slab[:, j * 512 : (j + 1) * 512], op=MAX,
                            )
                        nc.vector.tensor_reduce(
                            rowparts_s[:, rb : rb + 1], rowbuf[:],
                            axis=mybir.AxisListType.X, op=MAX,
                        )

            if repeat == 1:
                _body()
            else:
                # benchmark mode: body is idempotent (maxes), repeat on-device
                with tc.For_i(0, repeat, 1):
                    _body()

            nc.sync.dma_start(out=rowparts_d[:], in_=rowparts_s[:])
            nc.sync.dma_start(out=colmax_d[:], in_=colmax_s[:])

    nc.compile()
    return nc


def _build_nc_rt4(nc, repeat, batched_reduce=False, early_pools=False,
                  loop_hints=False):
    """Row-tiled variant: 4 concurrent K=5 matmuls in PE row-groups
    (tile_position=(32i,0)), processing 4 row blocks per group. Cuts PE
    busy time ~4x, which recovers DVE throughput (PE streaming measurably
    degrades concurrent DVE ops: 570 -> 1795 ns at full duty).

    Inputs: ap4 [128, _HALF//4] with row block 4k+i's lhsT at partitions
    32i..32i+4, cols k*128..; ag4 [128, _N] with ag replicated at
    partitions {32i..32i+4}.
    """
    import concourse.tile as tile
    from concourse import mybir

    f32 = mybir.dt.float32
    bf16 = mybir.dt.bfloat16
    MAX = mybir.AluOpType.max

    ap_d = nc.dram_tensor("ap4", [128, _HALF // 4], f32, kind="ExternalInput")
    ag_d = nc.dram_tensor("ag4", [128, _N], f32, kind="ExternalInput")
    rowparts_d = nc.dram_tensor("rowparts", [128, _RB], bf16, kind="ExternalOutput")
    colmax_d = nc.dram_tensor("colmax", [128, _N], bf16, kind="ExternalOutput")

    n_groups = _RB // 4  # 8 groups of 4 row blocks

    with tile.TileContext(nc) as tc:
        with (
            tc.tile_pool(name="singles", bufs=1) as singles,
            tc.tile_pool(name="slabs", bufs=2) as slabs,
            tc.tile_pool(name="rpool", bufs=2) as rpool,
            tc.tile_pool(name="psum", bufs=2, space="PSUM") as pp,
        ):
            ap_s = singles.tile([128, _HALF // 4], f32)
            nc.sync.dma_start(out=ap_s[:], in_=ap_d[:])
            ag_s = singles.tile([128, _N], f32)
            nc.sync.dma_start(out=ag_s[:], in_=ag_d[:])

            colmax_s = singles.tile([128, _N], bf16)
            nc.vector.memset(colmax_s[:], -_BIG)
            rowparts_s = singles.tile([128, _RB], bf16)
            if batched_reduce:
                parts_all = singles.tile([128, _RB * 8], bf16)

            def _body():
                for k in range(n_groups):
                    # slab4: 4 row blocks' staged tiles, each contiguous
                    # [128, 8192] region (i-major)
                    slab4 = slabs.tile([128, 4 * _N], bf16, tag="slab4")
                    if early_pools:
                        eparts = []
                        for i in range(4):
                            ep = rpool.tile([128, 8], bf16, tag=f"ep{i}")
                            eparts.append(ep)
                    for g in range(_N // 512):  # 16 column tiles
                        t = pp.tile([128, 2048], f32, tag="t")
                        for i in range(4):
                            nc.tensor.matmul(
                                t[:, i * 512 : (i + 1) * 512],
                                ap_s[32 * i : 32 * i + 5,
                                     k * 128 : (k + 1) * 128],
                                ag_s[32 * i : 32 * i + 5,
                                     g * 512 : (g + 1) * 512],
                                start=True,
                                stop=True,
                                tile_position=(32 * i, 0),
                            )
                        # scatter-exit: slice i -> row block i's slab region
                        dst = slab4[:].rearrange(
                            "p (i n) -> p i n", n=_N
                        )[:, :, g * 512 : (g + 1) * 512]
                        nc.scalar.copy(dst, t[:].rearrange(
                            "p (i n) -> p i n", n=512))
                        # column-max folds: 4 row blocks' contributions
                        cs = colmax_s[:, g * 512 : (g + 1) * 512]
                        for i in range(4):
                            src = slab4[:, i * _N + g * 512 : i * _N + (g + 1) * 512]
                            if early_pools and k == 0 and i == 0:
                                # first contribution: cheap 4x-mode copy
                                # instead of fold against memset
                                nc.vector.tensor_copy(cs, src)
                            else:
                                nc.vector.tensor_tensor(cs, cs, src, op=MAX)
                        if early_pools and g % 2 == 1:
                            # emit row-max pools as soon as their 1024-col
                            # chunk is staged (spreads pools through the
                            # group instead of bunching at the end)
                            q = g // 2
                            for i in range(4):
                                win = slab4[:, i * _N + q * 1024 :
                                            i * _N + (q + 1) * 1024].rearrange(
                                    "p (n w) -> p n w", w=1024
                                )
                                nc.vector.pool(
                                    eparts[i][:, q : q + 1], win,
                                    func=mybir.PoolFunctionType.max,
                                )
                    if early_pools:
                        for i in range(4):
                            nc.vector.tensor_reduce(
                                rowparts_s[:, 4 * k + i : 4 * k + i + 1],
                                eparts[i][:], axis=mybir.AxisListType.X,
                                op=MAX,
                            )
                        continue
                    # row max per row block (contiguous slab regions)
                    for i in range(4):
                        if batched_reduce:
                            parts = parts_all[:, (4 * k + i) * 8 :
                                              (4 * k + i) * 8 + 8]
                        else:
                            pt = rpool.tile([128, 8], bf16, tag="parts")
                            parts = pt[:]
                        for q in range(8):
                            win = slab4[:, i * _N + q * 1024 :
                                        i * _N + (q + 1) * 1024].rearrange(
                                "p (n w) -> p n w", w=1024
                            )
                            nc.vector.pool(
                                parts[:, q : q + 1], win,
                                func=mybir.PoolFunctionType.max,
                            )
                        if not batched_reduce:
                            nc.vector.tensor_reduce(
                                rowparts_s[:, 4 * k + i : 4 * k + i + 1],
                                parts, axis=mybir.AxisListType.X, op=MAX,
                            )
                if batched_reduce:
                    # one windowed reduce for all 32 row blocks
                    nc.vector.tensor_reduce(
                        rowparts_s[:],
                        parts_all[:].rearrange("p (rb q) -> p rb q", q=8),
                        axis=mybir.AxisListType.X, op=MAX,
                    )

            if repeat == 1:
                _body()
            elif loop_hints:
                with tc.For_i(0, repeat, 1, hint_engines=(
                        mybir.EngineType.PE, mybir.EngineType.DVE,
                        mybir.EngineType.Activation)):
                    _body()
            else:
                with tc.For_i(0, repeat, 1):
                    _body()

            nc.sync.dma_start(out=rowparts_d[:], in_=rowparts_s[:])
            nc.sync.dma_start(out=colmax_d[:], in_=colmax_s[:])

    nc.compile()
    return nc


def _prep_core_inputs_rt4(prediction, ground_truth):
    """Per-core inputs for the rt4 variant (row-group packed, negated)."""
    in_maps = []
    for c in range(_NCORES):
        b, h = divmod(c, 2)
        p = np.asarray(prediction[b, h * _HALF : (h + 1) * _HALF], dtype=np.float32)
        g = np.asarray(ground_truth[b], dtype=np.float32)
        ap = np.zeros((128, _HALF // 4), dtype=np.float32)
        # row block rb = 4k+i -> partitions 32i..32i+4, cols k*128..
        pts = p.reshape(_RB, 128, 3)  # [rb, c, 3]
        psq = (p * p).sum(axis=1, dtype=np.float32).reshape(_RB, 128)
        for i in range(4):
            blocks = np.arange(i, _RB, 4)  # rb = 4k+i for k=0..7
            seg = pts[blocks]              # [8, 128, 3]
            ap[32 * i + 0] = seg[:, :, 0].reshape(-1)
            ap[32 * i + 1] = seg[:, :, 1].reshape(-1)
            ap[32 * i + 2] = seg[:, :, 2].reshape(-1)
            ap[32 * i + 3] = psq[blocks].reshape(-1)
            ap[32 * i + 4] = 1.0
        ag = np.zeros((128, _N), dtype=np.float32)
        gsq = (g * g).sum(axis=1, dtype=np.float32)
        for i in range(4):
            ag[32 * i + 0 : 32 * i + 3] = (2.0 * g).T
            ag[32 * i + 3] = -1.0
            ag[32 * i + 4] = -gsq
        in_maps.append({"ap4": ap, "ag4": ag})
    return in_maps


def _get_nc():
    global _CACHED_NC
    if _CACHED_NC is None:
        _CACHED_NC = _build_nc()
    return _CACHED_NC


def _prep_core_inputs(prediction, ground_truth):
    """Build per-core inputs for the default variant."""
    if _DEFAULT_VARIANT.startswith("rt4"):
        return _prep_core_inputs_rt4(prediction, ground_truth)
    return _prep_core_inputs_std(prediction, ground_truth)


def _prep_core_inputs_std(prediction, ground_truth):
    """Build per-core augmented matrices (host-side, fp32), NEGATED form:
    ap.T @ ag = 2 p.g - ||p||^2 - ||g||^2 = -d2."""
    in_maps = []
    for c in range(_NCORES):
        b, h = divmod(c, 2)
        p = np.asarray(prediction[b, h * _HALF : (h + 1) * _HALF], dtype=np.float32)
        g = np.asarray(ground_truth[b], dtype=np.float32)
        ap = np.empty((5, _HALF), dtype=np.float32)
        ap[0:3] = p.T
        ap[3] = (p * p).sum(axis=1, dtype=np.float32)
        ap[4] = 1.0
        ag = np.empty((5, _N), dtype=np.float32)
        ag[0:3] = (2.0 * g).T
        ag[3] = -1.0
        ag[4] = -(g * g).sum(axis=1, dtype=np.float32)
        in_maps.append({"ap": ap, "ag": ag})
    return in_maps


def _make_runner(nc, n_cores):
    """Build a cached jitted SPMD executor for `nc` (axon/PJRT path).

    Mirrors concourse.bass2jax.run_bass_via_pjrt but caches the jitted
    callable so repeat calls don't re-trace/re-compile.
    """
    import jax
    import numpy as _np
    from jax.sharding import Mesh, PartitionSpec
    from jax.experimental.shard_map import shard_map
    from concourse import mybir
    from concourse.bass2jax import (
        _bass_exec_p,
        install_neuronx_cc_hook,
        partition_id_tensor,
    )

    install_neuronx_cc_hook()

    partition_name = (
        nc.partition_id_tensor.name if nc.partition_id_tensor else None
    )
    in_names, out_names, out_avals, zero_shapes = [], [], [], []
    for alloc in nc.m.functions[0].allocations:
        if not isinstance(alloc, mybir.MemoryLocationSet):
            continue
        name = alloc.memorylocations[0].name
        if alloc.kind == "ExternalInput":
            if name == partition_name:
                continue
            in_names.append(name)
        elif alloc.kind == "ExternalOutput":
            shape = tuple(alloc.tensor_shape)
            dtype = mybir.dt.np(alloc.dtype)
            out_names.append(name)
            out_avals.append(jax.core.ShapedArray(shape, dtype))
            zero_shapes.append((shape, dtype))
    n_params = len(in_names)
    n_outs = len(out_names)
    all_names = in_names + out_names
    if partition_name is not None:
        all_names = all_names + [partition_name]
    donate = tuple(range(n_params, n_params + n_outs))

    def _body(*args):
        operands = list(args)
        if partition_name is not None:
            operands.append(partition_id_tensor())
        outs = _bass_exec_p.bind(
            *operands,
            out_avals=tuple(out_avals),
            in_names=tuple(all_names),
            out_names=tuple(out_names),
            lowering_input_output_aliases=(),
            sim_require_finite=True,
            sim_require_nnan=True,
            nc=nc,
        )
        return tuple(outs)

    devices = jax.devices()[:n_cores]
    mesh = Mesh(_np.asarray(devices), ("core",))
    sharded = jax.jit(
        shard_map(
            _body,
            mesh=mesh,
            in_specs=(PartitionSpec("core"),) * (n_params + n_outs),
            out_specs=(PartitionSpec("core"),) * n_outs,
            check_rep=False,
        ),
        donate_argnums=donate,
        keep_unused=True,
    )

    def run(in_maps):
        concat_in = [
            _np.concatenate([m[name] for m in in_maps], axis=0)
            for name in in_names
        ]
        concat_zeros = [
            _np.zeros((n_cores * s[0], *s[1:]), d) for (s, d) in zero_shapes
        ]
        out_arrs = sharded(*concat_in, *concat_zeros)
        return [
            {
                name: _np.asarray(out_arrs[i]).reshape(
                    n_cores, *out_avals[i].shape
                )[c]
                for i, name in enumerate(out_names)
            }
            for c in range(n_cores)
        ]

    return run


def _get_runner(nc, n_cores=_NCORES):
    key = id(nc)
    if key not in _RUNNERS:
        _RUNNERS[key] = _make_runner(nc, n_cores)
    return _RUNNERS[key]


def kernel(prediction, ground_truth):
    prediction = np.asarray(prediction, dtype=np.float32)
    ground_truth = np.asarray(ground_truth, dtype=np.float32)

    nc = _get_nc()
    in_maps = _prep_core_inputs(prediction, ground_truth)
    results = _get_runner(nc)(in_maps)

    out = np.zeros(_B, dtype=np.float32)
    for b in range(_B):
        dx = 0.0
        cms = []
        for h in range(2):
            r = results[2 * b + h]
            # rowparts[p, rb] = max over cols of -d2 for row rb*128+p
            rp = r["rowparts"].astype(np.float32)  # [128, RB]
            dx += np.maximum(-rp, 0.0).sum(dtype=np.float64)
            # colmax[p, m] = max over this core's rows (partition p) of -d2
            cms.append(r["colmax"].astype(np.float32).max(axis=0))  # [N]
        cm = np.maximum(cms[0], cms[1])
        dy = np.maximum(-cm, 0.0).sum(dtype=np.float64)
        out[b] = dx / _N + dy / _N
    return out
